# revision 23
# baseline (speedup 1.0000x reference)
"""Trainium2 Bass kernel for CartesianDecomposedAttention (complex-valued attention).

Reference math (complex):
  Q = (x @ wq.T) * rotor ; K = (x @ wk.T) * rotor ; V = x @ wv.T
  scores = Q conj(K)^T / sqrt(Dh)
  attn_w = softmax(scores.re) * exp(i * scores.im)
  out    = (attn_w @ V) @ wo.T        -> stack([re, im], -1)

Sharding over 8 cores: core c -> batch b=c//4, head-group g=c%4 (4 heads, 256
model dims per group). Each core computes a partial output [S, D] (re+im);
the host sums the 4 group partials per batch (no on-device collectives).

Device decomposition (matmul operands fp16, PSUM accumulation fp32,
everything transposed so no on-chip transposes are needed):
  - Host ships pre-tiled operands that match the SBUF tile layouts exactly,
    so every DMA is a large contiguous-per-partition copy. x_re goes over the
    sync HWDGE queue, x_im over the scalar HWDGE queue (2x intake bandwidth),
    weights/tables over both plus the gpsimd SWDGE queues, in need-order.
  - RoPE via host-built cos/sin tables [128, S]; the 1/sqrt(Dh) score
    scale is folded into the Q-side tables. Stored components are chosen so
    both scoresT.re and scoresT.im are pure PSUM additions:
      Q: (Qr, Qin=-Qi)   K: (Kr, Kin=-Ki, Krn=-Kr)   V: (Vin=-Vi, Vr, Vi)
      Sr  = Kr.T Qr + Kin.T Qin          (= scores.re)
      Sip = Krn.T Qin + Kin.T Qr         (= +scores.im)
      attn_re = Vr.T ar + Vin.T ai ; attn_im = Vi.T ar + Vr.T ai
  - softmax without max-subtraction (scores in [-8, 8]); the denominator is
    a vector tree-add of the 8 exp tiles followed by one ones-column matmul
    per head (8 PE streams total instead of 64), applied *after* the AV
    matmul (1/r via reciprocal_approx_fast on a partition_broadcast tile).
  - HW Sin is valid only on [-pi, pi]: one add_range_wrap from the scores.im
    PSUM tile gives W = wrap(im); sin(im) = Sin(W) and
    cos(im) = cos(|W|) = Sin(-|W| + pi/2) via a cheap abs (tensor_scalar
    abs_max, 2x DVE mode) and the ACT pre-scale/bias -- no second wrap.
  - ACT exp and sin live in different table sets: chunks alternate
    exp-block / sin-block with explicit scheduling deps.
  - Chunk order (pair,s-half) = (0,0) (1,0) (0,1) (1,1): both pairs' s0
    attention finishes early, so the first half of the output projection
    overlaps the last chunk; only the second half trails.
  - V-projection PSUM copies run on gpsimd so they cannot stall the scalar
    (ACT) queue in front of the first sin block.
"""

import sys

for _p in ("/opt/trn_rl_repo",):
    if _p not in sys.path:
        sys.path.insert(0, _p)

import numpy as np
from contextlib import ExitStack

import concourse.bass as bass
import concourse.tile as tile
from concourse import bacc, mybir
from concourse.bass_utils import run_bass_kernel_spmd
from concourse.tile_rust import add_dep_helper

F32 = mybir.dt.float32
MM_DT = mybir.dt.float16          # matmul operand dtype
MM_NP = np.float16                # host-side dtype for matmul operands
TAB_DT = mybir.dt.float16         # rope table dtype
ALU = mybir.AluOpType
ACTF = mybir.ActivationFunctionType

D = 1024          # model dim
S = 1024          # sequence length
DH = 64           # head dim
JG = 256          # j-columns per head group (4 heads)
KT = 8            # k-tiles of 128 over D
PI = float(np.pi)


def _dep(frm, to, reason):
    """Scheduling-order dependency: `to` must come after `frm`.

    add_dep_helper's arg order is (waiter, prerequisite).
    """
    add_dep_helper(to.ins, frm.ins, sync=False, reason=reason)


def _build_kernel(tc, ins, outs):
    nc = tc.nc
    ctx = ExitStack()

    persist = ctx.enter_context(tc.tile_pool(name="persist", bufs=1))
    # shared PSUM pools: "mm" serves QKV projections, scores, the softmax
    # denominators and the output projection; "at" the AV accumulators.
    pmm = ctx.enter_context(tc.tile_pool(name="ps_mm", bufs=6, space="PSUM"))
    pat = ctx.enter_context(tc.tile_pool(name="ps_at", bufs=2, space="PSUM"))

    # --- persistent tensors (span phases) ---
    q_r = persist.tile([128, 2, S], MM_DT, name="q_r")     # [j%128, pair, s]
    q_in = persist.tile([128, 2, S], MM_DT, name="q_in")   # -(Q'im)
    k_r = persist.tile([128, 2, S], MM_DT, name="k_r")
    k_in = persist.tile([128, 2, S], MM_DT, name="k_in")   # -(K'im)
    k_rn = persist.tile([128, 2, S], MM_DT, name="k_rn")   # -(K're)
    v = persist.tile([128, KT, 3, JG], MM_DT, name="v")    # comps: (-Vi, Vr, Vi)
    attn_re = persist.tile([128, 2, S], MM_DT, name="attn_re")  # [j%128, pair, s]
    attn_im = persist.tile([128, 2, S], MM_DT, name="attn_im")
    ones_col = persist.tile([128, 1], MM_DT, name="ones_col")
    nc.vector.memset(ones_col, 1.0)

    wo = {}

    # phase-B pools (outer so they survive until the end of attention)
    pB = ExitStack()
    pE = pB.enter_context(tc.tile_pool(name="phB_E", bufs=8))
    pW = pB.enter_context(tc.tile_pool(name="phB_W", bufs=8))
    psm = pB.enter_context(tc.tile_pool(name="phB_sm", bufs=4))
    pbt = pB.enter_context(tc.tile_pool(name="phB_tmp", bufs=5))
    pes = pB.enter_context(tc.tile_pool(name="phB_es", bufs=1))

    # phase-A pools (inner; released after the V projection to make room)
    phA = ExitStack()
    pa = phA.enter_context(tc.tile_pool(name="phA", bufs=1))
    pw = phA.enter_context(tc.tile_pool(name="phA_w", bufs=1))

    x_re = pa.tile([128, KT, S], MM_DT, name="x_re", tag="x_re")
    x_im = pa.tile([128, KT, S], MM_DT, name="x_im", tag="x_im")
    w = {}  # (wname, pair, comp) -> [128, KT, 128] tile
    for wname in ("wq", "wk"):
        for pair in range(2):
            for comp in ("re", "im", "imn"):
                w[wname, pair, comp] = pw.tile(
                    [128, KT, 128], MM_DT, name=f"{wname}{pair}_{comp}",
                    tag=f"{wname}{pair}_{comp}")
    wv = pw.tile([128, KT, 3, JG], MM_DT, name="wv", tag="wv")
    tabs = {}
    for t in ("qc8", "qs8", "kcos", "ksin"):
        tabs[t] = pa.tile([128, S], TAB_DT, name=f"tab_{t}", tag=f"tab_{t}")

    # --- DMA issuance: sync carries x_re (+wq0 first, wv/wo after), scalar
    # carries x_im (+wk0 before the x s1-half, wq1/wk1 after), gpsimd SWDGE
    # carries the rope tables.  Slices ordered by first compute use.
    xr_d = ins["x_re"].rearrange("p (kt s) -> p kt s", kt=KT)
    xi_d = ins["x_im"].rearrange("p (kt s) -> p kt s", kt=KT)

    def w_dma(eng, wname, pair, comp):
        src = ins[f"{wname}{pair}T_{comp}"].rearrange("p (kt j) -> p kt j", kt=KT)
        eng.dma_start(out=w[wname, pair, comp], in_=src)

    # first-needed x chunks are small so the projection starts ~10us in;
    # wq0/wk0 + Q tables ride the gpsimd SWDGE queues in parallel
    nc.sync.dma_start(out=x_re[:, 0:1, 0:512], in_=xr_d[:, 0:1, 0:512])
    nc.sync.dma_start(out=x_re[:, 1:2, 0:512], in_=xr_d[:, 1:2, 0:512])
    nc.sync.dma_start(out=x_re[:, 2:4, 0:512], in_=xr_d[:, 2:4, 0:512])
    nc.sync.dma_start(out=x_re[:, 4:8, 0:512], in_=xr_d[:, 4:8, 0:512])
    nc.sync.dma_start(out=x_re[:, :, 512:1024], in_=xr_d[:, :, 512:1024])
    nc.sync.dma_start(out=tabs["kcos"], in_=ins["kcos"])
    nc.sync.dma_start(out=tabs["ksin"], in_=ins["ksin"])
    nc.sync.dma_start(
        out=wv, in_=ins["wvT"].rearrange("p (kt c j) -> p kt c j", kt=KT, c=3))

    nc.scalar.dma_start(out=x_im[:, 0:1, 0:512], in_=xi_d[:, 0:1, 0:512])
    nc.scalar.dma_start(out=x_im[:, 1:2, 0:512], in_=xi_d[:, 1:2, 0:512])
    nc.scalar.dma_start(out=x_im[:, 2:4, 0:512], in_=xi_d[:, 2:4, 0:512])
    nc.scalar.dma_start(out=x_im[:, 4:8, 0:512], in_=xi_d[:, 4:8, 0:512])
    nc.scalar.dma_start(out=x_im[:, :, 512:1024], in_=xi_d[:, :, 512:1024])
    for comp in ("re", "imn", "im"):
        w_dma(nc.scalar, "wq", 1, comp)
    for comp in ("re", "imn", "im"):
        w_dma(nc.scalar, "wk", 1, comp)

    for comp in ("re", "imn", "im"):
        w_dma(nc.gpsimd, "wq", 0, comp)
    for comp in ("re", "imn", "im"):
        w_dma(nc.gpsimd, "wk", 0, comp)
    nc.gpsimd.dma_start(out=tabs["qc8"], in_=ins["qc8"])
    nc.gpsimd.dma_start(out=tabs["qs8"], in_=ins["qs8"])

    def emit_qk(wname, pair, st, ctab, stab):
        """One s-half of one pair's Q/K projection + RoPE."""
        w_re = w[wname, pair, "re"]
        w_im = w[wname, pair, "im"]
        w_imn = w[wname, pair, "imn"]
        ssl = slice(st * 512, st * 512 + 512)
        ps_r = pmm.tile([128, 512], F32, name="ps_r", tag="mm")
        ps_i = pmm.tile([128, 512], F32, name="ps_i", tag="mm")
        for kt in range(KT):
            xr = x_re[:, kt, ssl]
            xi = x_im[:, kt, ssl]
            nc.tensor.matmul(ps_r, lhsT=w_re[:, kt, :], rhs=xr,
                             start=(kt == 0), stop=False)
            nc.tensor.matmul(ps_i, lhsT=w_re[:, kt, :], rhs=xi,
                             start=(kt == 0), stop=False)
            nc.tensor.matmul(ps_r, lhsT=w_imn[:, kt, :], rhs=xi,
                             start=False, stop=(kt == KT - 1))
            nc.tensor.matmul(ps_i, lhsT=w_im[:, kt, :], rhs=xr,
                             start=False, stop=(kt == KT - 1))
        # RoPE products: p1=Tr*c p2=Ti*s p3=Tr*s p4=Ti*c
        ct = tabs[ctab][:, ssl]
        st_t = tabs[stab][:, ssl]
        p1 = pbt.tile([128, 512], F32, name="p1", tag="ropetmp")
        p2 = pbt.tile([128, 512], F32, name="p2", tag="ropetmp")
        p3 = pbt.tile([128, 512], F32, name="p3", tag="ropetmp")
        p4 = pbt.tile([128, 512], F32, name="p4", tag="ropetmp")
        nc.vector.tensor_mul(p1, ps_r, ct)
        nc.vector.tensor_mul(p2, ps_i, st_t)
        nc.vector.tensor_mul(p3, ps_r, st_t)
        nc.vector.tensor_mul(p4, ps_i, ct)
        if wname == "wq":
            nc.vector.tensor_sub(q_r[:, pair, ssl], p1, p2)
            nc.vector.scalar_tensor_tensor(
                q_in[:, pair, ssl], in0=p3, scalar=-1.0, in1=p4,
                op0=ALU.mult, op1=ALU.subtract)
        else:
            nc.vector.tensor_sub(k_r[:, pair, ssl], p1, p2)
            nc.vector.tensor_sub(k_rn[:, pair, ssl], p2, p1)
            nc.vector.scalar_tensor_tensor(
                k_in[:, pair, ssl], in0=p3, scalar=-1.0, in1=p4,
                op0=ALU.mult, op1=ALU.subtract)

    def make_qk_pieces(wname, pair, ctab, stab):
        """Split one pair's projection into 2-kt pieces (both s-halves) so the
        PE work can be interleaved into a sin block's idle slots without
        blocking the in-order queues (ropes land after the sin block's own
        vector ops; at most two psum pairs are in flight)."""
        pieces = []
        for st in range(2):
            ssl = slice(st * 512, st * 512 + 512)
            hold = {}

            def piece(i, ssl=ssl, hold=hold, st=st):
                def f():
                    w_re = w[wname, pair, "re"]
                    w_im = w[wname, pair, "im"]
                    w_imn = w[wname, pair, "imn"]
                    if i == 0:
                        hold["r"] = pmm.tile([128, 512], F32, name="ps_r", tag="mm")
                        hold["i"] = pmm.tile([128, 512], F32, name="ps_i", tag="mm")
                    ps_r, ps_i = hold["r"], hold["i"]
                    for kt in range(2 * i, 2 * i + 2):
                        xr = x_re[:, kt, ssl]
                        xi = x_im[:, kt, ssl]
                        nc.tensor.matmul(ps_r, lhsT=w_re[:, kt, :], rhs=xr,
                                         start=(kt == 0), stop=False)
                        nc.tensor.matmul(ps_i, lhsT=w_re[:, kt, :], rhs=xi,
                                         start=(kt == 0), stop=False)
                        nc.tensor.matmul(ps_r, lhsT=w_imn[:, kt, :], rhs=xi,
                                         start=False, stop=(kt == KT - 1))
                        nc.tensor.matmul(ps_i, lhsT=w_im[:, kt, :], rhs=xr,
                                         start=False, stop=(kt == KT - 1))
                    if i == 3:
                        ct = tabs[ctab][:, ssl]
                        st_t = tabs[stab][:, ssl]
                        p1 = pbt.tile([128, 512], F32, name="p1", tag="ropetmp")
                        p2 = pbt.tile([128, 512], F32, name="p2", tag="ropetmp")
                        p3 = pbt.tile([128, 512], F32, name="p3", tag="ropetmp")
                        p4 = pbt.tile([128, 512], F32, name="p4", tag="ropetmp")
                        nc.vector.tensor_mul(p1, ps_r, ct)
                        nc.vector.tensor_mul(p2, ps_i, st_t)
                        nc.vector.tensor_mul(p3, ps_r, st_t)
                        nc.vector.tensor_mul(p4, ps_i, ct)
                        if wname == "wq":
                            nc.vector.tensor_sub(q_r[:, pair, ssl], p1, p2)
                            nc.vector.scalar_tensor_tensor(
                                q_in[:, pair, ssl], in0=p3, scalar=-1.0, in1=p4,
                                op0=ALU.mult, op1=ALU.subtract)
                        else:
                            nc.vector.tensor_sub(k_r[:, pair, ssl], p1, p2)
                            nc.vector.tensor_sub(k_rn[:, pair, ssl], p2, p1)
                            nc.vector.scalar_tensor_tensor(
                                k_in[:, pair, ssl], in0=p3, scalar=-1.0, in1=p4,
                                op0=ALU.mult, op1=ALU.subtract)
                return f

            for i in range(4):
                pieces.append(piece(i))
        return pieces

    def emit_v():
        """V-projection matmuls; the PSUM->SBUF copies are returned as
        per-tblk thunks so they can interleave with the first sin block's
        activations on the scalar queue (each V psum lands at ~3.5us cadence
        while scalar consumes sins+copies at ~3.7us/tblk)."""
        thunks = []
        for tblk in range(KT):
            ps_v = pmm.tile([128, 512], F32, name="ps_v", tag="mm")
            for kt in range(KT):
                lx_re = x_re[:, kt, tblk * 128:(tblk + 1) * 128]
                lx_im = x_im[:, kt, tblk * 128:(tblk + 1) * 128]
                nc.tensor.matmul(ps_v, lhsT=lx_re, rhs=wv[:, kt, 1:3, :],
                                 start=(kt == 0), stop=False)
                nc.tensor.matmul(ps_v, lhsT=lx_im, rhs=wv[:, kt, 0:2, :],
                                 start=False, stop=(kt == KT - 1))

            def cp(tblk=tblk, ps_v=ps_v):
                # copy out (ACT): v comps (0: -Vi, 1: Vr, 2: Vi)
                nc.scalar.copy(v[:, tblk, 1, :], ps_v[:, 0:256])
                nc.scalar.copy(v[:, tblk, 2, :], ps_v[:, 256:512])
                nc.scalar.activation(v[:, tblk, 0, :], ps_v[:, 256:512],
                                     ACTF.Copy, scale=-1.0)
            thunks.append(cp)
        return thunks

    state = {"prev_last_sin": None}

    def emit_scores(pair, sh, tblks=range(KT)):
        """Score matmuls for one chunk (or a tblk subrange).

        Emitted ahead of the previous chunk's sin block so ps_re is ready
        the moment the ACT table switches back to exp; the psum ring
        backpressures the PE until the exps/wraps start consuming.
        """
        ssl = slice(sh * 512, sh * 512 + 512)
        pss = {}
        for tblk in tblks:
            tsl = slice(tblk * 128, tblk * 128 + 128)
            pss[tblk] = [
                (pmm.tile([128, 512], F32, name="ps_sre", tag="mm"),
                 pmm.tile([128, 512], F32, name="ps_sip", tag="mm"))
                for hh in range(2)]
            # interleave hh so the two k=64 row-groups overlap on PE
            for chain in range(4):
                for hh in range(2):
                    dsl = slice(hh * 64, hh * 64 + 64)
                    ps_re, ps_ip = pss[tblk][hh]
                    lkr = k_r[dsl, pair, tsl]
                    lkin = k_in[dsl, pair, tsl]
                    lkrn = k_rn[dsl, pair, tsl]
                    rqr = q_r[dsl, pair, ssl]
                    rqin = q_in[dsl, pair, ssl]
                    if chain == 0:
                        nc.tensor.matmul(ps_re, lhsT=lkr, rhs=rqr,
                                         start=True, stop=False)
                    elif chain == 1:
                        nc.tensor.matmul(ps_ip, lhsT=lkrn, rhs=rqin,
                                         start=True, stop=False)
                    elif chain == 2:
                        nc.tensor.matmul(ps_re, lhsT=lkin, rhs=rqin,
                                         start=False, stop=True)
                    else:
                        nc.tensor.matmul(ps_ip, lhsT=lkin, rhs=rqr,
                                         start=False, stop=True)
        return pss

    def emit_eacts(pss, exp_insts, out=None):
        """Exp + range-wrap for previously emitted score psums."""
        E_tiles, W_tiles = out if out is not None else ([], [])
        for tblk in sorted(pss):
            Eb = pE.tile([128, 1024], MM_DT, name="Eb", tag="E")
            Wb = pW.tile([128, 1024], MM_DT, name="Wb", tag="W")
            for hh in range(2):
                hsl = slice(hh * 512, hh * 512 + 512)
                ps_re, ps_ip = pss[tblk][hh]
                ei = nc.scalar.activation(Eb[:, hsl], ps_re, ACTF.Exp)
                exp_insts.append(ei)
                if state["prev_last_sin"] is not None:
                    _dep(state["prev_last_sin"], ei, "act-table: exp after sins")
                nc.vector.add_range_wrap(Wb[:, hsl], ps_ip, shift=0.0,
                                         bound=PI, period=2.0 * PI)
            E_tiles.append(Eb)
            W_tiles.append(Wb)
        return (E_tiles, W_tiles)

    def emit_exp(pair, sh, exp_insts):
        return emit_eacts(emit_scores(pair, sh), exp_insts)

    def emit_sin(pair, sh, chunk_state, last_exp, filler=(), scalar_filler=()):
        filler = list(filler)
        scalar_filler = list(scalar_filler)
        ssl = slice(sh * 512, sh * 512 + 512)
        E_tiles, W_tiles = chunk_state
        at_re = pat.tile([128, 512], F32, name="at_re", tag="ps_at")
        at_im = pat.tile([128, 512], F32, name="at_im", tag="ps_at")
        es = None
        Rb = {}
        for tblk in range(KT):
            wc = pbt.tile([128, 1024], MM_DT, name="wc", tag="sintmp")
            nc.vector.add_range_wrap(wc, W_tiles[tblk], shift=PI / 2.0,
                                     bound=PI, period=2.0 * PI)
            cw = pbt.tile([128, 1024], MM_DT, name="cw", tag="sintmp")
            si1 = nc.scalar.activation(cw, wc, ACTF.Sin)  # cos(im)
            sw = pbt.tile([128, 1024], MM_DT, name="sw", tag="sintmp")
            si2 = nc.scalar.activation(sw, W_tiles[tblk], ACTF.Sin)  # sin(im)
            _dep(last_exp, si1, "act-table: sins after exps")
            _dep(last_exp, si2, "act-table: sins after exps")
            state["prev_last_sin"] = si2
            if scalar_filler:
                scalar_filler.pop(0)()
            ar = pbt.tile([128, 1024], MM_DT, name="ar", tag="avr")
            nc.vector.tensor_mul(ar, E_tiles[tblk], cw)         # exp*cos
            ai = pbt.tile([128, 1024], MM_DT, name="ai", tag="avr")
            nc.vector.tensor_mul(ai, E_tiles[tblk], sw)         # exp*sin
            # AV matmuls, hh-interleaved for col-group overlap
            for chain in range(4):
                for hh in range(2):
                    hsl = slice(hh * 512, hh * 512 + 512)
                    jsl = slice(pair * 128 + hh * 64, pair * 128 + hh * 64 + 64)
                    psl = slice(hh * 64, hh * 64 + 64)
                    lvin = v[:, tblk, 0, jsl]
                    lvr = v[:, tblk, 1, jsl]
                    lvi = v[:, tblk, 2, jsl]
                    if chain == 0:
                        nc.tensor.matmul(at_re[psl, :], lhsT=lvr, rhs=ar[:, hsl],
                                         start=(tblk == 0), stop=False)
                    elif chain == 1:
                        nc.tensor.matmul(at_im[psl, :], lhsT=lvi, rhs=ar[:, hsl],
                                         start=(tblk == 0), stop=False)
                    elif chain == 2:
                        nc.tensor.matmul(at_re[psl, :], lhsT=lvin, rhs=ai[:, hsl],
                                         start=False, stop=(tblk == KT - 1))
                    else:
                        nc.tensor.matmul(at_im[psl, :], lhsT=lvr, rhs=ai[:, hsl],
                                         start=False, stop=(tblk == KT - 1))
            if filler:
                filler.pop(0)()
            if tblk == 0:
                # softmax denominator: vector tree-add of the exp tiles,
                # emitted early in the sin block so it never gates the PE
                es = pes.tile([128, 1024], MM_DT, name="es", tag="es")
                nc.vector.tensor_add(es, E_tiles[0], E_tiles[1])
                for tb in range(2, KT):
                    nc.vector.tensor_add(es, es, E_tiles[tb])
            if tblk == 3:
                # one ones-column matmul per head on the summed tile (the es
                # chain has drained by now, so the PE does not wait)
                r_ps = pmm.tile([128, 512], F32, name="r_ps", tag="mm")
                for hh in range(2):
                    hsl = slice(hh * 512, hh * 512 + 512)
                    nc.tensor.matmul(r_ps[hh * 64:hh * 64 + 1, :],
                                     lhsT=ones_col, rhs=es[:, hsl],
                                     start=True, stop=True)
                    rrow = psm.tile([1, 512], F32, name="rrow", tag="rrow")
                    nc.vector.tensor_copy(rrow, r_ps[hh * 64:hh * 64 + 1, :])
                    rbraw = psm.tile([64, 512], F32, name="rbraw", tag="rbraw")
                    nc.gpsimd.partition_broadcast(rbraw, rrow)
                    rb = psm.tile([64, 512], F32, name="rb", tag="rb")
                    nc.vector.reciprocal_approx_fast(rb, rbraw)
                    Rb[hh] = rb
        # normalize + copy out
        for hh in range(2):
            psl = slice(hh * 64, hh * 64 + 64)
            jj = slice(hh * 64, hh * 64 + 64)
            nc.vector.tensor_mul(attn_re[jj, pair, ssl], at_re[psl, :], Rb[hh])
            nc.vector.tensor_mul(attn_im[jj, pair, ssl], at_im[psl, :], Rb[hh])

    def emit_out(po, lo, hi):
        for sblk in range(lo, hi):
            bsl = slice(sblk * 128, sblk * 128 + 128)
            o_r = po.tile([128, 1024], MM_DT, name="o_r", tag="otmp")
            o_i = po.tile([128, 1024], MM_DT, name="o_i", tag="otmp")
            for nt in range(2):
                nsl = slice(nt * 512, nt * 512 + 512)
                ps_or = pmm.tile([128, 512], F32, name="ps_or", tag="mm")
                ps_oi = pmm.tile([128, 512], F32, name="ps_oi", tag="mm")
                for kt in range(2):  # contraction over j (= pair dim)
                    la_r = attn_re[:, kt, bsl]
                    la_i = attn_im[:, kt, bsl]
                    nc.tensor.matmul(ps_or, lhsT=la_r, rhs=wo["T_re"][:, kt, nsl],
                                     start=(kt == 0), stop=False)
                    nc.tensor.matmul(ps_oi, lhsT=la_r, rhs=wo["T_im"][:, kt, nsl],
                                     start=(kt == 0), stop=False)
                    nc.tensor.matmul(ps_or, lhsT=la_i, rhs=wo["T_imn"][:, kt, nsl],
                                     start=False, stop=(kt == 1))
                    nc.tensor.matmul(ps_oi, lhsT=la_i, rhs=wo["T_re"][:, kt, nsl],
                                     start=False, stop=(kt == 1))
                nc.scalar.copy(o_r[:, nsl], ps_or)
                nc.vector.tensor_copy(o_i[:, nsl], ps_oi)
            eng = nc.sync if sblk % 2 == 0 else nc.scalar
            eng.dma_start(out=outs["out_re"][bsl, :], in_=o_r)
            nc.gpsimd.dma_start(out=outs["out_im"][bsl, :], in_=o_i)

    # =================== pipelined emission ===================
    # Chunk order (pair, s-half): (0,0) (1,0) (0,1) (1,1).  Per-chunk exp/sin
    # table phases alternate; the E-tile pool only holds one chunk, so a later
    # chunk's exps come after the previous chunk's sins (slot reuse) and the
    # scheduler overlaps the next chunk's score matmuls with the current
    # sin-block on the PE.  The s0 output projection is emitted after the last
    # exp block so it fills the PE during the final sin block.
    emit_qk("wq", 0, 0, "qc8", "qs8")
    emit_qk("wk", 0, 0, "kcos", "ksin")
    emit_qk("wq", 0, 1, "qc8", "qs8")
    emit_qk("wk", 0, 1, "kcos", "ksin")
    exps00 = []
    st00 = emit_exp(0, 0, exps00)
    v_copies = emit_v()
    emit_sin(0, 0, st00, exps00[-1],
             filler=make_qk_pieces("wq", 1, "qc8", "qs8"),
             scalar_filler=v_copies)

    exps01 = []
    st01 = emit_exp(0, 1, exps01)
    emit_sin(0, 1, st01, exps01[-1],
             filler=make_qk_pieces("wk", 1, "kcos", "ksin"))

    exps10 = []
    st10 = emit_exp(1, 0, exps10)

    phA.close()  # release x/weights space
    po_ctx = ExitStack()
    pcw = po_ctx.enter_context(tc.tile_pool(name="wo", bufs=1))
    for sfx in ("T_re", "T_im", "T_imn"):
        wo[sfx] = pcw.tile([128, 2, S], MM_DT, name=f"wo{sfx}", tag=f"wo{sfx}")
        nc.sync.dma_start(
            out=wo[sfx], in_=ins["wo" + sfx].rearrange("p (kt n) -> p kt n", kt=2))
    po = po_ctx.enter_context(tc.tile_pool(name="phC_o", bufs=8))
    emit_sin(1, 0, st10, exps10[-1])

    exps11 = []
    st11 = emit_exp(1, 1, exps11)
    emit_out(po, 0, 4)          # s0-half of the output projection
    emit_sin(1, 1, st11, exps11[-1])
    emit_out(po, 4, 8)          # s1-half trails

    po_ctx.close()
    pB.close()
    ctx.close()


_IN_SPECS = (
    [("x_re", [128, KT * S], MM_DT), ("x_im", [128, KT * S], MM_DT)]
    + [(f"{wn}{pair}T_{comp}", [128, KT * 128], MM_DT)
       for wn in ("wq", "wk") for pair in range(2)
       for comp in ("re", "im", "imn")]
    + [("wvT", [128, KT * 3 * JG], MM_DT)]
    + [("wo" + sfx, [128, 2 * S], MM_DT) for sfx in ("T_re", "T_im", "T_imn")]
    + [(t, [128, S], TAB_DT) for t in ("qc8", "qs8", "kcos", "ksin")]
)


def build_program():
    nc = bacc.Bacc("TRN2", target_bir_lowering=False, debug=False,
                   enable_asserts=False, num_devices=8)
    ins = {name: nc.dram_tensor(name, shape, dt, kind="ExternalInput").ap()
           for name, shape, dt in _IN_SPECS}
    outs = {name: nc.dram_tensor(name, [S, D], MM_DT, kind="ExternalOutput").ap()
            for name in ("out_re", "out_im")}
    with tile.TileContext(nc) as tc:
        _build_kernel(tc, ins, outs)
    nc.compile()
    return nc


def _make_tables():
    inv_freq = 1.0 / (10000.0 ** (np.arange(DH, dtype=np.float64) / DH))
    ang = np.arange(S, dtype=np.float64)[:, None] * inv_freq[None, :]  # [S, DH]
    angT = ang.T  # [DH, S]
    ang128 = np.concatenate([angT, angT], axis=0)  # [128, S]
    c = np.cos(ang128)
    s = np.sin(ang128)
    tab_np = np.float16
    return {
        "qc8": (c * 0.125).astype(tab_np),
        "qs8": (s * 0.125).astype(tab_np),
        "kcos": c.astype(tab_np),
        "ksin": s.astype(tab_np),
    }


def _tile_kt(a, kt=KT):
    """[kt*128, N] -> [128, kt*N]: per-partition-contiguous SBUF layout."""
    n = a.shape[1]
    return np.ascontiguousarray(
        a.reshape(kt, 128, n).transpose(1, 0, 2).reshape(128, kt * n))


def _core_inputs(inputs, c, tables):
    b, g = divmod(c, 4)
    rows = slice(g * JG, (g + 1) * JG)

    def f(a):
        return np.ascontiguousarray(np.asarray(a, dtype=np.float32)).astype(MM_NP)

    m = {
        "x_re": _tile_kt(f(np.asarray(inputs["x_re"])[b].T)),
        "x_im": _tile_kt(f(np.asarray(inputs["x_im"])[b].T)),
    }
    woT_re = f(np.asarray(inputs["wo_re"])[:, rows].T)   # [256, 1024]
    woT_im = f(np.asarray(inputs["wo_im"])[:, rows].T)
    m["woT_re"] = _tile_kt(woT_re, kt=2)
    m["woT_im"] = _tile_kt(woT_im, kt=2)
    m["woT_imn"] = _tile_kt(-woT_im, kt=2)
    for wn in ("wq", "wk"):
        wre = f(np.asarray(inputs[wn + "_re"])[rows]).T   # [1024, 256]
        wim = f(np.asarray(inputs[wn + "_im"])[rows]).T
        for pair in range(2):
            jsl = slice(pair * 128, pair * 128 + 128)
            m[f"{wn}{pair}T_re"] = _tile_kt(wre[:, jsl])
            m[f"{wn}{pair}T_im"] = _tile_kt(wim[:, jsl])
            m[f"{wn}{pair}T_imn"] = _tile_kt(-wim[:, jsl])
    vre = f(np.asarray(inputs["wv_re"])[rows]).T          # [1024, 256]
    vim = f(np.asarray(inputs["wv_im"])[rows]).T
    # comps in free dim: 0=-Vim, 1=Vre, 2=Vim (so rhs slices [1:3] and [0:2]
    # pair with x_re / x_im as in emit_v)
    wv3 = np.stack([-vim, vre, vim], axis=1)              # [1024, 3, 256]
    m["wvT"] = _tile_kt(wv3.reshape(D, 3 * JG))
    m.update(tables)
    return m


_PROGRAM = None


def _get_program():
    global _PROGRAM
    if _PROGRAM is None:
        _PROGRAM = build_program()
    return _PROGRAM


def run(inputs, trace=False, **kwargs):
    nc = _get_program()
    tables = _make_tables()
    in_maps = [_core_inputs(inputs, c, tables) for c in range(8)]
    res = run_bass_kernel_spmd(nc, in_maps, list(range(8)), trace=trace, **kwargs)
    B = 2
    out = np.zeros((B, S, D, 2), np.float32)
    for c, r in enumerate(res.results):
        b = c // 4
        out[b, :, :, 0] += r["out_re"].astype(np.float32)
        out[b, :, :, 1] += r["out_im"].astype(np.float32)
    return out, res


def kernel(**inputs):
    out, _ = run(inputs)
    return out


if __name__ == "__main__":
    nc = build_program()
    print("program built + compiled OK")
